# revision 1
# baseline (speedup 1.0000x reference)
"""Trainium2 Bass kernel for nn_Decoder (embed -> LSTM -> vocab projection).

v2 layout (8 NeuronCores, single SPMD NEFF):
  - Host: embedding gather + concat -> lstm_in; pre-transpose weights.
  - gx GEMM in rows-layout: gx[320,4096] = x @ W_ih^T + b (x stationary,
    W_ih^T moving, bias added on PSUM eviction).
  - LSTM recurrence, data-parallel over batch (32/core): gates[32,4096]
    computed with h_T as the stationary operand (64 N=512 matmuls/step) and
    the gx contribution folded in via an identity-matmul partition-select.
    ScalarE applies sigmoid/tanh straight from PSUM. h is re-transposed
    each step on the PE (h_T feeds the next step + the FC lhsT).
  - Per-timestep AllGather of h_T (runs on TOPSP, overlapped with compute).
  - FC vocab-sharded: logits[2560,3750] = hs @ fc_W_shard^T + fc_b, rows
    processed in 2 halves to fit SBUF; 1280 N<=512 fp32r matmuls.
  - Host: undo row permutation, concat vocab shards.

All matmuls are float32r (tf32-class, ~1.5e-4): raw fp32 bits are DMA'd
directly into float32r tensors (no on-device rounding pass needed).
"""
import ml_dtypes
import numpy as np
import jax
from jax.sharding import Mesh, PartitionSpec
from jax.experimental.shard_map import shard_map

import concourse.bass as bass
import concourse.mybir as mybir
import concourse.tile as tile
from concourse import bacc
from concourse.bass2jax import _bass_exec_p, install_neuronx_cc_hook, partition_id_tensor
from concourse.masks import make_identity

P = 128
NCORES = 8
B, T, FEAT, EMB, HID, VOCAB = 256, 10, 512, 512, 1024, 30000
DIN = FEAT + EMB          # 1024
G = 4 * HID               # 4096
BL = B // NCORES          # 32 batches per core
RL = BL * T               # 320 rows per core (t-major: r = t*BL + b)
RA = RL * NCORES          # 2560 rows total
VL = VOCAB // NCORES      # 3750 vocab per core
KT = DIN // P             # 8 contraction tiles
NCH = G // 512            # 8 gate column chunks
F32 = mybir.dt.float32
F32R = mybir.dt.float32r
BF16 = mybir.dt.bfloat16
Act = mybir.ActivationFunctionType

_CACHE = {}


def _build_nc():
    nc = bacc.Bacc("TRN2", target_bir_lowering=False, debug=False, num_devices=NCORES)
    x_T = nc.dram_tensor("x_T", [DIN, RL], BF16, kind="ExternalInput").ap()
    w_ih_T = nc.dram_tensor("w_ih_T", [DIN, G], BF16, kind="ExternalInput").ap()
    w_hh_T = nc.dram_tensor("w_hh_T", [HID, G], BF16, kind="ExternalInput").ap()
    bias_rep = nc.dram_tensor("bias_rep", [P, G], F32, kind="ExternalInput").ap()
    fc_wT = nc.dram_tensor("fc_wT", [HID, VL], BF16, kind="ExternalInput").ap()
    fc_b_rep = nc.dram_tensor("fc_b_rep", [P, VL], F32, kind="ExternalInput").ap()
    logits = nc.dram_tensor("logits", [RA, VL], F32, kind="ExternalOutput").ap()

    MT_X = [(0, 128), (128, 128), (256, 64)]  # (row0, rows) m-tiles of 320
    WINS = [(0, 1024), (1024, 1024), (2048, 1024), (3072, 678)]
    NORD = [0, 2, 4, 6, 1, 3, 5, 7]  # gate chunks: h-half0 first, then half1

    with tile.TileContext(nc) as tc:
        with tc.tile_pool(name="dram", bufs=1, space="DRAM") as dram_pool:
            hs_dram = dram_pool.tile([T, HID, BL], BF16)
            ag_outs = [dram_pool.tile([NCORES, HID, BL], BF16,
                                      addr_space="Shared", name=f"ag_{t}")
                       for t in range(T)]
            gx_dram = dram_pool.tile([3, P, G], BF16)

            with tc.tile_pool(name="persist", bufs=1) as persist, \
                 tc.tile_pool(name="phD", bufs=1) as phD, \
                 tc.tile_pool(name="hsT_pool", bufs=2) as hsT_pool, \
                 tc.tile_pool(name="fcw_pool", bufs=2) as fcw_pool, \
                 tc.tile_pool(name="fc_out", bufs=3) as fc_out:
                ident_f = persist.tile([P, P], F32)
                make_identity(nc, ident_f[:])
                ident_b = persist.tile([P, P], BF16)
                nc.vector.tensor_copy(ident_b[:], ident_f[:])
                gx_t0 = persist.tile([32, G], BF16)
                fcb_sb = phD.tile([P, VL], F32)
                GSZ = [512, 512, 256]   # rows per t-group (t0-3, t4-7, t8-9)
                GT0 = [0, 4, 8]
                hsT_tiles = {}
                for rh in range(2):
                    for g in range(3):
                        hsT_tiles[rh, g] = hsT_pool.tile(
                            [P, KT, GSZ[g]], BF16,
                            name=f"hsT_{rh}_{g}", tag=f"hsT_{g}")

                GB = [0, 512, 1024]

                def fc_block(rh, v0, vn, fw, g, ml, psum_pool, tagsfx=""):
                    row0 = rh * (RA // 2) + GB[g] + ml * P
                    hsT_sb = hsT_tiles[rh, g]
                    for n0 in range(0, vn, 512):
                        nsz = min(512, vn - n0)
                        ps = psum_pool.tile(
                            [P, 512], F32,
                            name=f"fps{tagsfx}_{rh}_{v0}_{g}_{ml}_{n0}",
                            tag=f"fps{tagsfx}", bufs=2 if tagsfx else None)
                        for k in range(KT):
                            nc.tensor.matmul(
                                ps[:, 0:nsz],
                                hsT_sb[:, k, ml * P:(ml + 1) * P],
                                fw[:, k, n0:n0 + nsz],
                                start=(k == 0), stop=(k == KT - 1))
                        ot = fc_out.tile(
                            [P, 512], F32,
                            name=f"fo_{rh}_{v0}_{g}_{ml}_{n0}", tag="fo")
                        nc.vector.tensor_add(
                            ot[:, 0:nsz], ps[:, 0:nsz],
                            fcb_sb[:, v0 + n0:v0 + n0 + nsz])
                        nc.scalar.dma_start(
                            logits[row0:row0 + P, v0 + n0:v0 + n0 + nsz],
                            ot[:, 0:nsz])

                # ---- Phase A: gx = x @ W_ih^T + b   (rows x gates, bf16) ----
                with tc.tile_pool(name="phA", bufs=1) as phA, \
                     tc.tile_pool(name="wih_pool", bufs=3) as wih_pool, \
                     tc.tile_pool(name="gx_stage", bufs=4) as gx_stage, \
                     tc.tile_pool(name="gx_psum", bufs=4, space="PSUM") as gx_psum:
                    x_sb = phA.tile([P, KT, RL], BF16)
                    for k in range(KT):
                        nc.scalar.dma_start(
                            x_sb[:, k, :], x_T[k * P:(k + 1) * P, :])
                    bias_sb = phA.tile([P, G], F32)
                    nc.scalar.dma_start(bias_sb[:], bias_rep)
                    for n in range(NCH):
                        wt = wih_pool.tile([P, KT, 512], BF16,
                                           name=f"wih_{n}", tag="wih")
                        for kk in range(0, KT, 2):
                            nc.sync.dma_start(
                                wt[:, kk:kk + 2, :],
                                w_ih_T[:, n * 512:(n + 1) * 512].rearrange(
                                    "(k p) v -> p k v", p=P)[:, kk:kk + 2, :])
                        for mi, (r0, rn) in enumerate(MT_X):
                            ps = gx_psum.tile([P, 512], F32,
                                              name=f"gxps_{n}_{mi}", tag="gxps")
                            for k in range(KT):
                                nc.tensor.matmul(
                                    ps[0:rn, :], x_sb[:, k, r0:r0 + rn],
                                    wt[:, k, :],
                                    start=(k == 0), stop=(k == KT - 1))
                            gt = gx_stage.tile([P, 512], BF16,
                                               name=f"gxs_{n}_{mi}", tag="gxs")
                            nc.vector.tensor_add(
                                gt[0:rn, :],
                                ps[0:rn, :], bias_sb[0:rn, n * 512:(n + 1) * 512])
                            if mi == 0:
                                # keep t=0's rows on-chip (skips DRAM roundtrip)
                                nc.vector.tensor_copy(
                                    gx_t0[:, n * 512:(n + 1) * 512], gt[0:32, :])
                            nc.scalar.dma_start(
                                gx_dram[mi, 0:rn, n * 512:(n + 1) * 512], gt[0:rn, :])

                nc.scalar.dma_start(fcb_sb[:], fc_b_rep)

                # ---- Phase B: LSTM recurrence (bf16) ----
                with tc.tile_pool(name="phB", bufs=1) as phB, \
                     tc.tile_pool(name="whh_pool", bufs=1) as whh_pool, \
                     tc.tile_pool(name="gxb_pool", bufs=1) as gxb_pool, \
                     tc.tile_pool(name="gch_psum", bufs=4, space="PSUM") as gch_psum, \
                     tc.tile_pool(name="tp_psum", bufs=2, space="PSUM") as tp_psum, \
                     tc.tile_pool(name="step_pool", bufs=1) as step_pool:
                    whh_sb = whh_pool.tile([P, KT, G], BF16)
                    for n in range(NCH):
                        for kk in range(0, KT, 4):
                            nc.gpsimd.dma_start(
                                whh_sb[:, kk:kk + 4, n * 512:(n + 1) * 512],
                                w_hh_T[:, n * 512:(n + 1) * 512].rearrange(
                                    "(k p) v -> p k v", p=P)[:, kk:kk + 4, :])
                    h_Tb = phB.tile([P, KT, BL], BF16)
                    c_sb = phB.tile([32, HID], F32)
                    fw0 = fcw_pool.tile([P, KT, 1024], BF16, name="fcw_w0", tag="fcw")
                    for kk in range(0, KT, 4):
                        nc.sync.dma_start(
                            fw0[:, kk:kk + 4, :],
                            fc_wT[:, 0:1024].rearrange(
                                "(k p) v -> p k v", p=P)[:, kk:kk + 4, :])

                    for t in range(T):
                        mt, j = t // 4, t % 4
                        if t > 0:
                            gxt = gxb_pool.tile([32, G], BF16,
                                                name=f"gxt_{t}", tag="gxt")
                            nc.scalar.dma_start(
                                gxt[:], gx_dram[mt, 32 * j:32 * j + 32, :])
                        gates4 = step_pool.tile([32, 4, HID], F32,
                                                name=f"gates_{t}", tag="gates")
                        tmp = step_pool.tile([32, HID], F32, name=f"tmp_{t}", tag="tmp")
                        th = step_pool.tile([32, HID], F32, name=f"th_{t}", tag="th")
                        h_sb = step_pool.tile([32, HID], BF16, name=f"h_{t}", tag="h")

                        def gate_chunk(n):
                            g4, half = n // 2, n % 2
                            dst = gates4[:, g4, half * 512:(half + 1) * 512]
                            if t == 0:
                                nc.scalar.activation(
                                    dst, gx_t0[:, n * 512:(n + 1) * 512],
                                    Act.Tanh if g4 == 2 else Act.Sigmoid)
                                return
                            ps = gch_psum.tile([32, 512], F32,
                                               name=f"gps_{t}_{n}", tag="gps")
                            nc.tensor.matmul(
                                ps[:], ident_b[0:32, 0:32],
                                gxt[:, n * 512:(n + 1) * 512],
                                start=True, stop=False)
                            for k in range(KT):
                                nc.tensor.matmul(
                                    ps[:], h_Tb[:, k, :],
                                    whh_sb[:, k, n * 512:(n + 1) * 512],
                                    start=False, stop=(k == KT - 1))
                            nc.scalar.activation(
                                dst, ps[:], Act.Tanh if g4 == 2 else Act.Sigmoid)

                        def half_elemwise(half):
                            sl = slice(half * 512, (half + 1) * 512)
                            nc.vector.tensor_mul(tmp[:, sl], gates4[:, 0, sl],
                                                 gates4[:, 2, sl])
                            if t == 0:
                                nc.vector.tensor_copy(c_sb[:, sl], tmp[:, sl])
                            else:
                                nc.vector.tensor_mul(c_sb[:, sl], gates4[:, 1, sl],
                                                     c_sb[:, sl])
                                nc.vector.tensor_add(c_sb[:, sl], c_sb[:, sl],
                                                     tmp[:, sl])
                            nc.scalar.activation(th[:, sl], c_sb[:, sl], Act.Tanh)
                            nc.vector.tensor_mul(h_sb[:, sl], gates4[:, 3, sl],
                                                 th[:, sl])

                        for n in NORD[:4]:
                            gate_chunk(n)
                        half_elemwise(0)
                        for n in NORD[4:]:
                            gate_chunk(n)
                        half_elemwise(1)
                        for k in range(KT):
                            tp = tp_psum.tile([P, 32], BF16,
                                              name=f"tp_{t}_{k}", tag="tp")
                            nc.tensor.transpose(
                                tp[:], h_sb[:, k * P:(k + 1) * P], ident_b[0:32, 0:32])
                            nc.vector.tensor_copy(h_Tb[:, k, :], tp[:])
                        nc.scalar.dma_start(
                            hs_dram[t].rearrange("(k p) b -> p k b", p=P), h_Tb[:])
                        nc.gpsimd.collective_compute(
                            "AllGather", mybir.AluOpType.bypass,
                            replica_groups=[list(range(NCORES))],
                            ins=[hs_dram[t].opt()], outs=[ag_outs[t].opt()])
                        tg = 0 if t < 4 else (1 if t < 8 else 2)
                        ntg = 4 if tg < 2 else 2
                        for a in range(NCORES):
                            rh = a // 4
                            r0 = (a % 4) * ntg * BL + (t - GT0[tg]) * BL
                            nc.gpsimd.dma_start(
                                hsT_tiles[rh, tg][:, :, r0:r0 + BL],
                                ag_outs[t][a].rearrange("(k p) b -> p k b", p=P))
                        # backfill PE stalls with early FC work (fw0 resident)
                        ILV = {6: [(0, 0)], 7: [(0, 1)],
                               8: [(0, 2), (0, 3)], 9: [(1, 0), (1, 1)]}
                        for (g, ml) in ILV.get(t, []):
                            fc_block(0, 0, 1024, fw0, g, ml, gch_psum, tagsfx="i")

                # ---- Phase D: FC; t-groups 0/1 first, g2 (needs AG_9) last ----
                with tc.tile_pool(name="fc_psum", bufs=6, space="PSUM") as fc_psum:
                    def load_fw(tag_name, v0, vn):
                        fw = fcw_pool.tile([P, KT, 1024], BF16,
                                           name=tag_name, tag="fcw")
                        for kk in range(0, KT, 4):
                            nc.sync.dma_start(
                                fw[:, kk:kk + 4, 0:vn],
                                fc_wT[:, v0:v0 + vn].rearrange(
                                    "(k p) v -> p k v", p=P)[:, kk:kk + 4, :])
                        return fw

                    for rh in range(2):
                        for wi, (v0, vn) in enumerate(WINS):
                            fw = fw0 if (rh == 0 and wi == 0) \
                                else load_fw(f"fcw_{rh}_{v0}", v0, vn)
                            done = {(0, 0), (0, 1), (0, 2), (0, 3),
                                    (1, 0), (1, 1)} if (rh == 0 and wi == 0) else set()
                            for g in (0, 1):
                                for ml in range(4):
                                    if (g, ml) in done or (g == 1 and ml >= 4):
                                        continue
                                    fc_block(rh, v0, vn, fw, g, ml, fc_psum)
                    for rh in range(2):
                        for wi, (v0, vn) in enumerate(WINS):
                            fw = load_fw(f"fcwg2_{rh}_{v0}", v0, vn)
                            for ml in range(2):
                                fc_block(rh, v0, vn, fw, 2, ml, fc_psum)
    nc.compile()
    return nc


def _build_sharded(nc, n_cores=NCORES):
    install_neuronx_cc_hook()
    partition_name = nc.partition_id_tensor.name if nc.partition_id_tensor else None
    in_names, out_names, out_avals, zero_shapes = [], [], [], []
    for alloc in nc.m.functions[0].allocations:
        if not isinstance(alloc, mybir.MemoryLocationSet):
            continue
        name = alloc.memorylocations[0].name
        if alloc.kind == "ExternalInput":
            if name != partition_name:
                in_names.append(name)
        elif alloc.kind == "ExternalOutput":
            out_names.append(name)
            shape = tuple(alloc.tensor_shape)
            dtype = mybir.dt.np(alloc.dtype)
            out_avals.append(jax.core.ShapedArray(shape, dtype))
            zero_shapes.append((shape, dtype))
    n_params = len(in_names)
    n_outs = len(out_avals)
    all_in_names = list(in_names) + list(out_names)
    if partition_name is not None:
        all_in_names.append(partition_name)
    donate = tuple(range(n_params, n_params + n_outs))

    def _body(*args):
        operands = list(args)
        if partition_name is not None:
            operands.append(partition_id_tensor())
        outs = _bass_exec_p.bind(
            *operands,
            out_avals=tuple(out_avals),
            in_names=tuple(all_in_names),
            out_names=tuple(out_names),
            lowering_input_output_aliases=(),
            sim_require_finite=True,
            sim_require_nnan=True,
            nc=nc,
        )
        return tuple(outs)

    devices = jax.devices("axon")[:n_cores]
    mesh = Mesh(np.asarray(devices), ("core",))
    in_specs = (PartitionSpec("core"),) * (n_params + n_outs)
    out_specs = (PartitionSpec("core"),) * len(out_names)
    sharded = jax.jit(
        shard_map(_body, mesh=mesh, in_specs=in_specs, out_specs=out_specs,
                  check_rep=False),
        donate_argnums=donate, keep_unused=True)

    def run(in_maps):
        concat_in = [
            np.concatenate([np.asarray(m[name]) for m in in_maps], axis=0)
            for name in in_names
        ]
        concat_zeros = [np.zeros((n_cores * s[0], *s[1:]), d) for s, d in zero_shapes]
        out_arrs = sharded(*concat_in, *concat_zeros)
        jax.block_until_ready(out_arrs)
        return [
            {name: np.asarray(out_arrs[i]).reshape(n_cores, *out_avals[i].shape)[c]
             for i, name in enumerate(out_names)}
            for c in range(n_cores)
        ]

    return run


def _prep_inputs(features, captions, emb_table, W_ih, W_hh, b_ih, b_hh, fc_W, fc_b):
    features = np.asarray(features, dtype=np.float32)
    captions = np.asarray(captions)
    emb_table = np.asarray(emb_table, dtype=np.float32)
    W_ih = np.asarray(W_ih, dtype=np.float32)
    W_hh = np.asarray(W_hh, dtype=np.float32)
    b = (np.asarray(b_ih, dtype=np.float32) + np.asarray(b_hh, dtype=np.float32))
    fc_W = np.asarray(fc_W, dtype=np.float32)
    fc_b = np.asarray(fc_b, dtype=np.float32)

    embedded = emb_table[captions.astype(np.int64)]          # [B, T, EMB]
    lstm_in = np.concatenate([features, embedded], axis=-1)  # [B, T, DIN]

    w_ih_T = np.ascontiguousarray(W_ih.T.astype(ml_dtypes.bfloat16))
    w_hh_T = np.ascontiguousarray(W_hh.T.astype(ml_dtypes.bfloat16))
    bias_rep = np.ascontiguousarray(np.broadcast_to(b, (P, G)))

    in_maps = []
    for c in range(NCORES):
        xc = lstm_in[c * BL:(c + 1) * BL]                    # [BL, T, DIN]
        x_T = np.ascontiguousarray(
            xc.transpose(2, 1, 0).reshape(DIN, RL).astype(ml_dtypes.bfloat16))
        fc_wT = np.ascontiguousarray(
            fc_W[c * VL:(c + 1) * VL].T.astype(ml_dtypes.bfloat16))
        fcb_rep = np.ascontiguousarray(
            np.broadcast_to(fc_b[c * VL:(c + 1) * VL], (P, VL)))
        in_maps.append({
            "x_T": x_T, "w_ih_T": w_ih_T, "w_hh_T": w_hh_T, "bias_rep": bias_rep,
            "fc_wT": fc_wT, "fc_b_rep": fcb_rep,
        })
    return in_maps


def _row_perm():
    # device row r' -> (batch b_global, t); build gather index: perm[b*T+t] = r'
    perm = np.empty(B * T, dtype=np.int64)
    GT0 = [0, 4, 8]
    GSZ = [512, 512, 256]
    for rh in range(2):
        base_rh = rh * (RA // 2)
        for tg in range(3):
            ntg = 4 if tg < 2 else 2
            gbase = base_rh + sum(GSZ[:tg])
            for ap in range(4):
                for trem in range(ntg):
                    t = GT0[tg] + trem
                    for b in range(BL):
                        bg = (rh * 4 + ap) * BL + b
                        perm[bg * T + t] = gbase + ap * ntg * BL + trem * BL + b
    return perm


_PERM = _row_perm()


def _unshard(results):
    out = np.empty((B, T, VOCAB), dtype=np.float32)
    for c in range(NCORES):
        lg = results[c]["logits"][_PERM]                     # [B*T, VL]
        out[:, :, c * VL:(c + 1) * VL] = lg.reshape(B, T, VL)
    return out


def kernel(features, captions, emb_table, W_ih, W_hh, b_ih, b_hh, fc_W, fc_b):
    if "nc" not in _CACHE:
        _CACHE["nc"] = _build_nc()
    if "run" not in _CACHE:
        _CACHE["run"] = _build_sharded(_CACHE["nc"])
    in_maps = _prep_inputs(features, captions, emb_table, W_ih, W_hh, b_ih, b_hh,
                           fc_W, fc_b)
    results = _CACHE["run"](in_maps)
    return _unshard(results)


def kernel_traced(features, captions, emb_table, W_ih, W_hh, b_ih, b_hh, fc_W, fc_b):
    """Same computation via run_bass_kernel_spmd(trace=True); returns
    (output, BassKernelResults) so the caller can read exec_time_ns."""
    from concourse.bass_utils import run_bass_kernel_spmd
    if "nc" not in _CACHE:
        _CACHE["nc"] = _build_nc()
    in_maps = _prep_inputs(features, captions, emb_table, W_ih, W_hh, b_ih, b_hh,
                           fc_W, fc_b)
    res = run_bass_kernel_spmd(_CACHE["nc"], in_maps, list(range(NCORES)), trace=True)
    return _unshard(res.results), res



# revision 3
# speedup vs baseline: 1.2764x; 1.2764x over previous
"""Trainium2 Bass kernel for nn_Decoder (embed -> LSTM -> vocab projection).

v3 layout (8 NeuronCores, single SPMD NEFF):
  - Host: embedding gather + concat -> lstm_in; pre-transpose weights.
  - LSTM recurrence is tensor-parallel over the HIDDEN dim: each core owns
    128 hidden units (512 gate rows i|f|g|o) and the full batch (256).
    Gates are computed TRANSPOSED, gates_T[512, 256] = Wc^T-chunks.T @
    [x_t; h_{t-1}]_T, so every matmul runs at full M=128 PE efficiency,
    the x@W_ih contribution is fused into the same k-loop (k=16 tiles of
    128), the per-gate bias rides the ScalarE activation's per-partition
    bias port, and h_T[128, 256] falls out of the elementwise ops already
    transposed -- no PE transposes, no gx phase, no DRAM roundtrip.
  - Per-step AllGather of h_T chunks (64KB/core) rebuilds the full
    h_T[1024, 256] on every core: next step's moving operand AND the FC
    lhsT come straight from the gathered tile.
  - FC vocab-sharded as before: logits[2560, 3750] = hs @ fc_W_shard^T
    + fc_b, but interleaved ONE STEP BEHIND the recurrence: after step
    t's gate matmuls the PE runs FC for step t-1 (~31us), which hides
    the ~18us AllGather latency completely.
  - logits are written bf16 (halves the 38MB output DMA); host casts to
    f32. All matmuls bf16 (1 cycle/col on the PE).
"""
import ml_dtypes
import numpy as np
import jax
from jax.sharding import Mesh, PartitionSpec
from jax.experimental.shard_map import shard_map

import concourse.bass as bass
import concourse.mybir as mybir
import concourse.tile as tile
from concourse import bacc
from concourse.bass2jax import _bass_exec_p, install_neuronx_cc_hook, partition_id_tensor

P = 128
NCORES = 8
B, T, FEAT, EMB, HID, VOCAB = 256, 10, 512, 512, 1024, 30000
DIN = FEAT + EMB          # 1024
KX = DIN // P             # 8 x-contraction tiles
KH = HID // P             # 8 h-contraction tiles
KT = KX + KH              # 16
HL = HID // NCORES        # 128 hidden units per core
GC = 4 * HL               # 512 gate rows per core (i|f|g|o)
VL = VOCAB // NCORES      # 3750 vocab per core
RA = B * T                # 2560 logit rows (r = t*B + b)
F32 = mybir.dt.float32
BF16 = mybir.dt.bfloat16
Act = mybir.ActivationFunctionType

_CACHE = {}


def _build_nc():
    nc = bacc.Bacc("TRN2", target_bir_lowering=False, debug=False, num_devices=NCORES)
    x_T = nc.dram_tensor("x_T", [DIN, RA], BF16, kind="ExternalInput").ap()
    wc_T = nc.dram_tensor("wc_T", [KT * P, GC], BF16, kind="ExternalInput").ap()
    bias4 = nc.dram_tensor("bias4", [P, 4], F32, kind="ExternalInput").ap()
    fc_wT = nc.dram_tensor("fc_wT", [HID, VL], BF16, kind="ExternalInput").ap()
    fc_b_rep = nc.dram_tensor("fc_b_rep", [P, VL], F32, kind="ExternalInput").ap()
    logits = nc.dram_tensor("logits", [RA, VL], BF16, kind="ExternalOutput").ap()

    GFUNC = [Act.Sigmoid, Act.Sigmoid, Act.Tanh, Act.Sigmoid]  # i, f, g, o

    with tile.TileContext(nc) as tc:
        with tc.tile_pool(name="dram", bufs=1, space="DRAM") as dram_pool:
            h_loc = dram_pool.tile([T, P, B], BF16)
            ag_outs = [dram_pool.tile([NCORES * P, B], BF16,
                                      addr_space="Shared", name=f"ag_{t}")
                       for t in range(T)]

            with tc.tile_pool(name="persist", bufs=1) as persist, \
                 tc.tile_pool(name="hT_pool", bufs=3) as hT_pool, \
                 tc.tile_pool(name="act_pool", bufs=2) as act_pool, \
                 tc.tile_pool(name="fc_out", bufs=4) as fc_out, \
                 tc.tile_pool(name="gpsum", bufs=1, space="PSUM") as gpsum, \
                 tc.tile_pool(name="fpsum", bufs=3, space="PSUM") as fpsum:
                wc_sb = persist.tile([P, KT, GC], BF16)
                x_sb = persist.tile([P, KX, RA], BF16)
                bias_sb = persist.tile([P, 4], F32)
                fcw_sb = persist.tile([P, KH, VL], BF16)
                fcb_sb = persist.tile([P, VL], F32)
                c_sb = persist.tile([P, B], F32)

                # --- preloads (split across queues; first-needed first) ---
                nc.sync.dma_start(
                    wc_sb[:], wc_T.rearrange("(k p) g -> p k g", p=P))
                xr = x_T.rearrange("(k p) r -> p k r", p=P)
                nc.scalar.dma_start(x_sb[:, :, 0:B], xr[:, :, 0:B])
                nc.scalar.dma_start(bias_sb[:], bias4)
                nc.scalar.dma_start(x_sb[:, :, B:RA], xr[:, :, B:RA])
                nc.sync.dma_start(fcb_sb[:], fc_b_rep)
                fwr = fc_wT.rearrange("(k p) v -> p k v", p=P)
                for n0 in range(0, VL, 512):
                    nsz = min(512, VL - n0)
                    nc.gpsimd.dma_start(
                        fcw_sb[:, :, n0:n0 + nsz], fwr[:, :, n0:n0 + nsz])

                hT_tiles = {}

                def fc_step(s):
                    hs = hT_tiles[s]
                    for mh in range(2):
                        r0 = s * B + mh * P
                        for n0 in range(0, VL, 512):
                            nsz = min(512, VL - n0)
                            ps = fpsum.tile([P, 512], F32,
                                            name=f"fps_{s}_{mh}_{n0}", tag="fps")
                            for k in range(KH):
                                nc.tensor.matmul(
                                    ps[:, 0:nsz],
                                    hs[:, k, mh * P:(mh + 1) * P],
                                    fcw_sb[:, k, n0:n0 + nsz],
                                    start=(k == 0), stop=(k == KH - 1))
                            ot = fc_out.tile([P, 512], BF16,
                                             name=f"fo_{s}_{mh}_{n0}", tag="fo")
                            nc.vector.tensor_add(
                                ot[:, 0:nsz], ps[:, 0:nsz],
                                fcb_sb[:, n0:n0 + nsz])
                            nc.scalar.dma_start(
                                logits[r0:r0 + P, n0:n0 + nsz], ot[:, 0:nsz])

                for t in range(T):
                    qs = (0, 2, 3) if t == 0 else (0, 1, 2, 3)
                    nk = KX if t == 0 else KT
                    act4 = act_pool.tile([P, 4, B], F32,
                                         name=f"act_{t}", tag="act")
                    for q in qs:
                        ps = gpsum.tile([P, B], F32,
                                        name=f"gps_{t}_{q}", tag=f"gps{q}")
                        for k in range(nk):
                            rhs = (x_sb[:, k, t * B:(t + 1) * B] if k < KX
                                   else hT_tiles[t - 1][:, k - KX, :])
                            nc.tensor.matmul(
                                ps[:], wc_sb[:, k, q * HL:(q + 1) * HL], rhs,
                                start=(k == 0), stop=(k == nk - 1))
                        nc.scalar.activation(
                            act4[:, q, :], ps[:], GFUNC[q],
                            bias=bias_sb[:, q:q + 1])
                    th = act_pool.tile([P, B], F32, name=f"th_{t}", tag="th")
                    h_sb = act_pool.tile([P, B], BF16, name=f"h_{t}", tag="h")
                    if t == 0:
                        nc.vector.tensor_mul(c_sb[:], act4[:, 0, :], act4[:, 2, :])
                    else:
                        tmp = act_pool.tile([P, B], F32,
                                            name=f"tmp_{t}", tag="tmp")
                        nc.vector.tensor_mul(tmp[:], act4[:, 0, :], act4[:, 2, :])
                        nc.vector.tensor_mul(c_sb[:], act4[:, 1, :], c_sb[:])
                        nc.vector.tensor_add(c_sb[:], c_sb[:], tmp[:])
                    nc.scalar.activation(th[:], c_sb[:], Act.Tanh)
                    nc.vector.tensor_mul(h_sb[:], act4[:, 3, :], th[:])
                    nc.sync.dma_start(h_loc[t], h_sb[:])
                    nc.gpsimd.collective_compute(
                        "AllGather", mybir.AluOpType.bypass,
                        replica_groups=[list(range(NCORES))],
                        ins=[h_loc[t].opt()], outs=[ag_outs[t].opt()])
                    hT = hT_pool.tile([P, KH, B], BF16,
                                      name=f"hT_{t}", tag="hT")
                    nc.sync.dma_start(
                        hT[:], ag_outs[t].rearrange("(j p) b -> p j b", p=P))
                    hT_tiles[t] = hT
                    if t >= 1:
                        fc_step(t - 1)
                fc_step(T - 1)
    nc.compile()
    return nc


def _build_sharded(nc, n_cores=NCORES):
    install_neuronx_cc_hook()
    partition_name = nc.partition_id_tensor.name if nc.partition_id_tensor else None
    in_names, out_names, out_avals, zero_shapes = [], [], [], []
    for alloc in nc.m.functions[0].allocations:
        if not isinstance(alloc, mybir.MemoryLocationSet):
            continue
        name = alloc.memorylocations[0].name
        if alloc.kind == "ExternalInput":
            if name != partition_name:
                in_names.append(name)
        elif alloc.kind == "ExternalOutput":
            out_names.append(name)
            shape = tuple(alloc.tensor_shape)
            dtype = mybir.dt.np(alloc.dtype)
            out_avals.append(jax.core.ShapedArray(shape, dtype))
            zero_shapes.append((shape, dtype))
    n_params = len(in_names)
    n_outs = len(out_avals)
    all_in_names = list(in_names) + list(out_names)
    if partition_name is not None:
        all_in_names.append(partition_name)
    donate = tuple(range(n_params, n_params + n_outs))

    def _body(*args):
        operands = list(args)
        if partition_name is not None:
            operands.append(partition_id_tensor())
        outs = _bass_exec_p.bind(
            *operands,
            out_avals=tuple(out_avals),
            in_names=tuple(all_in_names),
            out_names=tuple(out_names),
            lowering_input_output_aliases=(),
            sim_require_finite=True,
            sim_require_nnan=True,
            nc=nc,
        )
        return tuple(outs)

    devices = jax.devices("axon")[:n_cores]
    mesh = Mesh(np.asarray(devices), ("core",))
    in_specs = (PartitionSpec("core"),) * (n_params + n_outs)
    out_specs = (PartitionSpec("core"),) * len(out_names)
    sharded = jax.jit(
        shard_map(_body, mesh=mesh, in_specs=in_specs, out_specs=out_specs,
                  check_rep=False),
        donate_argnums=donate, keep_unused=True)

    def run(in_maps):
        concat_in = [
            np.concatenate([np.asarray(m[name]) for m in in_maps], axis=0)
            for name in in_names
        ]
        concat_zeros = [np.zeros((n_cores * s[0], *s[1:]), d) for s, d in zero_shapes]
        out_arrs = sharded(*concat_in, *concat_zeros)
        jax.block_until_ready(out_arrs)
        return [
            {name: np.asarray(out_arrs[i]).reshape(n_cores, *out_avals[i].shape)[c]
             for i, name in enumerate(out_names)}
            for c in range(n_cores)
        ]

    return run


def _prep_inputs(features, captions, emb_table, W_ih, W_hh, b_ih, b_hh, fc_W, fc_b):
    features = np.asarray(features, dtype=np.float32)
    captions = np.asarray(captions)
    emb_table = np.asarray(emb_table, dtype=np.float32)
    W_ih = np.asarray(W_ih, dtype=np.float32)
    W_hh = np.asarray(W_hh, dtype=np.float32)
    b = (np.asarray(b_ih, dtype=np.float32) + np.asarray(b_hh, dtype=np.float32))
    fc_W = np.asarray(fc_W, dtype=np.float32)
    fc_b = np.asarray(fc_b, dtype=np.float32)

    embedded = emb_table[captions.astype(np.int64)]          # [B, T, EMB]
    lstm_in = np.concatenate([features, embedded], axis=-1)  # [B, T, DIN]
    # x_T[din, t*B + b]
    x_T = np.ascontiguousarray(
        lstm_in.transpose(2, 1, 0).reshape(DIN, RA).astype(ml_dtypes.bfloat16))
    wcat = np.concatenate([W_ih, W_hh], axis=1)              # [4H, 2048]
    wcat4 = wcat.reshape(4, HID, KT * P)
    b4 = b.reshape(4, NCORES, HL)

    in_maps = []
    for c in range(NCORES):
        # per-core gate rows (its 128 hidden units x 4 gates), transposed:
        # wc_T[kin, q*HL + j] = wcat[q*HID + c*HL + j, kin]
        wc = wcat4[:, c * HL:(c + 1) * HL, :]                # [4, HL, 2048]
        wc_T = np.ascontiguousarray(
            wc.transpose(2, 0, 1).reshape(KT * P, GC).astype(ml_dtypes.bfloat16))
        bias4 = np.ascontiguousarray(b4[:, c, :].T)          # [HL, 4] f32
        fc_wT = np.ascontiguousarray(
            fc_W[c * VL:(c + 1) * VL].T.astype(ml_dtypes.bfloat16))
        fcb_rep = np.ascontiguousarray(
            np.broadcast_to(fc_b[c * VL:(c + 1) * VL], (P, VL)))
        in_maps.append({
            "x_T": x_T, "wc_T": wc_T, "bias4": bias4,
            "fc_wT": fc_wT, "fc_b_rep": fcb_rep,
        })
    return in_maps


def _unshard(results):
    out = np.empty((B, T, VOCAB), dtype=np.float32)
    for c in range(NCORES):
        lg = np.asarray(results[c]["logits"]).astype(np.float32)  # [RA, VL]
        out[:, :, c * VL:(c + 1) * VL] = lg.reshape(T, B, VL).transpose(1, 0, 2)
    return out


def kernel(features, captions, emb_table, W_ih, W_hh, b_ih, b_hh, fc_W, fc_b):
    if "nc" not in _CACHE:
        _CACHE["nc"] = _build_nc()
    if "run" not in _CACHE:
        _CACHE["run"] = _build_sharded(_CACHE["nc"])
    in_maps = _prep_inputs(features, captions, emb_table, W_ih, W_hh, b_ih, b_hh,
                           fc_W, fc_b)
    results = _CACHE["run"](in_maps)
    return _unshard(results)


def kernel_traced(features, captions, emb_table, W_ih, W_hh, b_ih, b_hh, fc_W, fc_b):
    """Same computation via run_bass_kernel_spmd(trace=True); returns
    (output, BassKernelResults) so the caller can read exec_time_ns."""
    from concourse.bass_utils import run_bass_kernel_spmd
    if "nc" not in _CACHE:
        _CACHE["nc"] = _build_nc()
    in_maps = _prep_inputs(features, captions, emb_table, W_ih, W_hh, b_ih, b_hh,
                           fc_W, fc_b)
    res = run_bass_kernel_spmd(_CACHE["nc"], in_maps, list(range(NCORES)), trace=True)
    return _unshard(res.results), res


# revision 6
# speedup vs baseline: 1.3020x; 1.0201x over previous
"""Trainium2 Bass kernel for nn_Decoder (embed -> LSTM -> vocab projection).

v3 layout (8 NeuronCores, single SPMD NEFF):
  - Host: embedding gather + concat -> lstm_in; pre-transpose weights.
  - LSTM recurrence is tensor-parallel over the HIDDEN dim: each core owns
    128 hidden units (512 gate rows i|f|g|o) and the full batch (256).
    Gates are computed TRANSPOSED, gates_T[512, 256] = Wc^T-chunks.T @
    [x_t; h_{t-1}]_T, so every matmul runs at full M=128 PE efficiency,
    the x@W_ih contribution is fused into the same k-loop (k=16 tiles of
    128), the per-gate bias rides the ScalarE activation's per-partition
    bias port, and h_T[128, 256] falls out of the elementwise ops already
    transposed -- no PE transposes, no gx phase, no DRAM roundtrip.
  - Per-step AllGather of h_T chunks (64KB/core) rebuilds the full
    h_T[1024, 256] on every core: next step's moving operand AND the FC
    lhsT come straight from the gathered tile.
  - FC vocab-sharded as before: logits[2560, 3750] = hs @ fc_W_shard^T
    + fc_b, but interleaved ONE STEP BEHIND the recurrence: after step
    t's gate matmuls the PE runs FC for step t-1 (~31us), which hides
    the ~18us AllGather latency completely.
  - logits are written bf16 (halves the 38MB output DMA); host casts to
    f32. All matmuls bf16 (1 cycle/col on the PE).
"""
import ml_dtypes
import numpy as np
import jax
from jax.sharding import Mesh, PartitionSpec
from jax.experimental.shard_map import shard_map

import concourse.bass as bass
import concourse.mybir as mybir
import concourse.tile as tile
from concourse import bacc
from concourse.bass2jax import _bass_exec_p, install_neuronx_cc_hook, partition_id_tensor

P = 128
NCORES = 8
B, T, FEAT, EMB, HID, VOCAB = 256, 10, 512, 512, 1024, 30000
DIN = FEAT + EMB          # 1024
KX = DIN // P             # 8 x-contraction tiles
KH = HID // P             # 8 h-contraction tiles
KT = KX + KH              # 16
HL = HID // NCORES        # 128 hidden units per core
GC = 4 * HL               # 512 gate rows per core (i|f|g|o)
VL = VOCAB // NCORES      # 3750 vocab per core
RA = B * T                # 2560 logit rows (r = t*B + b)
F32 = mybir.dt.float32
BF16 = mybir.dt.bfloat16
Act = mybir.ActivationFunctionType

_CACHE = {}


def _build_nc():
    nc = bacc.Bacc("TRN2", target_bir_lowering=False, debug=False, num_devices=NCORES)
    x_T = nc.dram_tensor("x_T", [DIN, RA], BF16, kind="ExternalInput").ap()
    wc_T = nc.dram_tensor("wc_T", [KT * P, GC], BF16, kind="ExternalInput").ap()
    bias4 = nc.dram_tensor("bias4", [P, 4], F32, kind="ExternalInput").ap()
    fc_wT = nc.dram_tensor("fc_wT", [HID, VL], BF16, kind="ExternalInput").ap()
    fc_b_rep = nc.dram_tensor("fc_b_rep", [P, VL], F32, kind="ExternalInput").ap()
    logits = nc.dram_tensor("logits", [RA, VL], BF16, kind="ExternalOutput").ap()

    GFUNC = [Act.Sigmoid, Act.Sigmoid, Act.Tanh, Act.Sigmoid]  # i, f, g, o

    with tile.TileContext(nc) as tc:
        with tc.tile_pool(name="dram", bufs=1, space="DRAM") as dram_pool:
            h_loc = dram_pool.tile([T, P, B], BF16)
            ag_outs = [dram_pool.tile([NCORES * P, B], BF16,
                                      addr_space="Shared", name=f"ag_{t}")
                       for t in range(T)]
            warm_in = dram_pool.tile([P, 16], BF16)
            warm_out = dram_pool.tile([NCORES * P, 16], BF16,
                                      addr_space="Shared", name="ag_warm")

            with tc.tile_pool(name="persist", bufs=1) as persist, \
                 tc.tile_pool(name="hT_pool", bufs=3) as hT_pool, \
                 tc.tile_pool(name="act_pool", bufs=2) as act_pool, \
                 tc.tile_pool(name="fc_out", bufs=4) as fc_out, \
                 tc.tile_pool(name="gpsum", bufs=1, space="PSUM") as gpsum, \
                 tc.tile_pool(name="fpsum", bufs=3, space="PSUM") as fpsum:
                wc_sb = persist.tile([P, KT, GC], BF16)
                x_sb = persist.tile([P, KX, RA], BF16)
                bias_sb = persist.tile([P, 4], F32)
                fcw_sb = persist.tile([P, KH, VL], BF16)
                fcb_sb = persist.tile([P, VL], F32)
                c_sb = persist.tile([P, B], F32)

                # --- warm-up AllGather: absorb the ~20us cold-start of the
                # collective path while phase-1 preloads stream ---
                warm_sb = persist.tile([P, 16], BF16)
                nc.gpsimd.memset(warm_sb[:], 0.0)
                nc.gpsimd.dma_start(warm_in[:], warm_sb[:])
                nc.gpsimd.collective_compute(
                    "AllGather", mybir.AluOpType.bypass,
                    replica_groups=[list(range(NCORES))],
                    ins=[warm_in.opt()], outs=[warm_out.opt()])

                # --- phase-1 preloads: just what step 0/1 matmuls need.
                # Everything else is deferred to after step 0's activations
                # so it can't steal HBM bandwidth from wc/x0 (DMA rings
                # fair-share; 17MB up-front made the first matmul wait 43us).
                nc.sync.dma_start(
                    wc_sb[:], wc_T.rearrange("(k p) g -> p k g", p=P))
                xr = x_T.rearrange("(k p) r -> p k r", p=P)
                nc.scalar.dma_start(x_sb[:, :, 0:2 * B], xr[:, :, 0:2 * B])
                nc.scalar.dma_start(bias_sb[:], bias4)

                def phase2_preloads():
                    nc.scalar.dma_start(fcb_sb[:], fc_b_rep)
                    nc.scalar.dma_start(x_sb[:, :, 2 * B:RA], xr[:, :, 2 * B:RA])
                    fwr = fc_wT.rearrange("(k p) v -> p k v", p=P)
                    for n0 in range(0, VL, 512):
                        nsz = min(512, VL - n0)
                        # 1-col overlap with the previous window creates a
                        # WAW chain so windows complete in the order the FC
                        # consumes them (instead of fair-sharing to finish
                        # simultaneously)
                        s0 = max(0, n0 - 1)
                        nc.scalar.dma_start(
                            fcw_sb[:, :, s0:n0 + nsz], fwr[:, :, s0:n0 + nsz])

                hT_tiles = {}

                def fc_step(s):
                    hs = hT_tiles[s]
                    for mh in range(2):
                        r0 = s * B + mh * P
                        for n0 in range(0, VL, 512):
                            nsz = min(512, VL - n0)
                            ps = fpsum.tile([P, 512], F32,
                                            name=f"fps_{s}_{mh}_{n0}", tag="fps")
                            for k in range(KH):
                                nc.tensor.matmul(
                                    ps[:, 0:nsz],
                                    hs[:, k, mh * P:(mh + 1) * P],
                                    fcw_sb[:, k, n0:n0 + nsz],
                                    start=(k == 0), stop=(k == KH - 1))
                            ot = fc_out.tile([P, 512], BF16,
                                             name=f"fo_{s}_{mh}_{n0}", tag="fo")
                            nc.vector.tensor_add(
                                ot[:, 0:nsz], ps[:, 0:nsz],
                                fcb_sb[:, n0:n0 + nsz])
                            nc.scalar.dma_start(
                                logits[r0:r0 + P, n0:n0 + nsz], ot[:, 0:nsz])

                for t in range(T):
                    qs = (0, 2, 3) if t == 0 else (0, 1, 2, 3)
                    nk = KX if t == 0 else KT
                    act4 = act_pool.tile([P, 4, B], F32,
                                         name=f"act_{t}", tag="act")
                    for q in qs:
                        ps = gpsum.tile([P, B], F32,
                                        name=f"gps_{t}_{q}", tag=f"gps{q}")
                        for k in range(nk):
                            rhs = (x_sb[:, k, t * B:(t + 1) * B] if k < KX
                                   else hT_tiles[t - 1][:, k - KX, :])
                            nc.tensor.matmul(
                                ps[:], wc_sb[:, k, q * HL:(q + 1) * HL], rhs,
                                start=(k == 0), stop=(k == nk - 1))
                        nc.scalar.activation(
                            act4[:, q, :], ps[:], GFUNC[q],
                            bias=bias_sb[:, q:q + 1])
                    if t == 0:
                        phase2_preloads()
                    th = act_pool.tile([P, B], F32, name=f"th_{t}", tag="th")
                    h_sb = act_pool.tile([P, B], BF16, name=f"h_{t}", tag="h")
                    if t == 0:
                        nc.vector.tensor_mul(c_sb[:], act4[:, 0, :], act4[:, 2, :])
                    else:
                        tmp = act_pool.tile([P, B], F32,
                                            name=f"tmp_{t}", tag="tmp")
                        nc.vector.tensor_mul(tmp[:], act4[:, 0, :], act4[:, 2, :])
                        nc.vector.tensor_mul(c_sb[:], act4[:, 1, :], c_sb[:])
                        nc.vector.tensor_add(c_sb[:], c_sb[:], tmp[:])
                    nc.scalar.activation(th[:], c_sb[:], Act.Tanh)
                    nc.vector.tensor_mul(h_sb[:], act4[:, 3, :], th[:])
                    nc.sync.dma_start(h_loc[t], h_sb[:])
                    nc.gpsimd.collective_compute(
                        "AllGather", mybir.AluOpType.bypass,
                        replica_groups=[list(range(NCORES))],
                        ins=[h_loc[t].opt()], outs=[ag_outs[t].opt()])
                    hT = hT_pool.tile([P, KH, B], BF16,
                                      name=f"hT_{t}", tag="hT")
                    nc.sync.dma_start(
                        hT[:], ag_outs[t].rearrange("(j p) b -> p j b", p=P))
                    hT_tiles[t] = hT
                    if t >= 1:
                        fc_step(t - 1)
                fc_step(T - 1)
    nc.compile()
    return nc


def _build_sharded(nc, n_cores=NCORES):
    install_neuronx_cc_hook()
    partition_name = nc.partition_id_tensor.name if nc.partition_id_tensor else None
    in_names, out_names, out_avals, zero_shapes = [], [], [], []
    for alloc in nc.m.functions[0].allocations:
        if not isinstance(alloc, mybir.MemoryLocationSet):
            continue
        name = alloc.memorylocations[0].name
        if alloc.kind == "ExternalInput":
            if name != partition_name:
                in_names.append(name)
        elif alloc.kind == "ExternalOutput":
            out_names.append(name)
            shape = tuple(alloc.tensor_shape)
            dtype = mybir.dt.np(alloc.dtype)
            out_avals.append(jax.core.ShapedArray(shape, dtype))
            zero_shapes.append((shape, dtype))
    n_params = len(in_names)
    n_outs = len(out_avals)
    all_in_names = list(in_names) + list(out_names)
    if partition_name is not None:
        all_in_names.append(partition_name)
    donate = tuple(range(n_params, n_params + n_outs))

    def _body(*args):
        operands = list(args)
        if partition_name is not None:
            operands.append(partition_id_tensor())
        outs = _bass_exec_p.bind(
            *operands,
            out_avals=tuple(out_avals),
            in_names=tuple(all_in_names),
            out_names=tuple(out_names),
            lowering_input_output_aliases=(),
            sim_require_finite=True,
            sim_require_nnan=True,
            nc=nc,
        )
        return tuple(outs)

    devices = jax.devices("axon")[:n_cores]
    mesh = Mesh(np.asarray(devices), ("core",))
    in_specs = (PartitionSpec("core"),) * (n_params + n_outs)
    out_specs = (PartitionSpec("core"),) * len(out_names)
    sharded = jax.jit(
        shard_map(_body, mesh=mesh, in_specs=in_specs, out_specs=out_specs,
                  check_rep=False),
        donate_argnums=donate, keep_unused=True)

    def run(in_maps):
        concat_in = [
            np.concatenate([np.asarray(m[name]) for m in in_maps], axis=0)
            for name in in_names
        ]
        concat_zeros = [np.zeros((n_cores * s[0], *s[1:]), d) for s, d in zero_shapes]
        out_arrs = sharded(*concat_in, *concat_zeros)
        jax.block_until_ready(out_arrs)
        return [
            {name: np.asarray(out_arrs[i]).reshape(n_cores, *out_avals[i].shape)[c]
             for i, name in enumerate(out_names)}
            for c in range(n_cores)
        ]

    return run


def _prep_inputs(features, captions, emb_table, W_ih, W_hh, b_ih, b_hh, fc_W, fc_b):
    features = np.asarray(features, dtype=np.float32)
    captions = np.asarray(captions)
    emb_table = np.asarray(emb_table, dtype=np.float32)
    W_ih = np.asarray(W_ih, dtype=np.float32)
    W_hh = np.asarray(W_hh, dtype=np.float32)
    b = (np.asarray(b_ih, dtype=np.float32) + np.asarray(b_hh, dtype=np.float32))
    fc_W = np.asarray(fc_W, dtype=np.float32)
    fc_b = np.asarray(fc_b, dtype=np.float32)

    embedded = emb_table[captions.astype(np.int64)]          # [B, T, EMB]
    lstm_in = np.concatenate([features, embedded], axis=-1)  # [B, T, DIN]
    # x_T[din, t*B + b]
    x_T = np.ascontiguousarray(
        lstm_in.transpose(2, 1, 0).reshape(DIN, RA).astype(ml_dtypes.bfloat16))
    wcat = np.concatenate([W_ih, W_hh], axis=1)              # [4H, 2048]
    wcat4 = wcat.reshape(4, HID, KT * P)
    b4 = b.reshape(4, NCORES, HL)

    in_maps = []
    for c in range(NCORES):
        # per-core gate rows (its 128 hidden units x 4 gates), transposed:
        # wc_T[kin, q*HL + j] = wcat[q*HID + c*HL + j, kin]
        wc = wcat4[:, c * HL:(c + 1) * HL, :]                # [4, HL, 2048]
        wc_T = np.ascontiguousarray(
            wc.transpose(2, 0, 1).reshape(KT * P, GC).astype(ml_dtypes.bfloat16))
        bias4 = np.ascontiguousarray(b4[:, c, :].T)          # [HL, 4] f32
        fc_wT = np.ascontiguousarray(
            fc_W[c * VL:(c + 1) * VL].T.astype(ml_dtypes.bfloat16))
        fcb_rep = np.ascontiguousarray(
            np.broadcast_to(fc_b[c * VL:(c + 1) * VL], (P, VL)))
        in_maps.append({
            "x_T": x_T, "wc_T": wc_T, "bias4": bias4,
            "fc_wT": fc_wT, "fc_b_rep": fcb_rep,
        })
    return in_maps


def _unshard(results):
    out = np.empty((B, T, VOCAB), dtype=np.float32)
    for c in range(NCORES):
        lg = np.asarray(results[c]["logits"]).astype(np.float32)  # [RA, VL]
        out[:, :, c * VL:(c + 1) * VL] = lg.reshape(T, B, VL).transpose(1, 0, 2)
    return out


def kernel(features, captions, emb_table, W_ih, W_hh, b_ih, b_hh, fc_W, fc_b):
    if "nc" not in _CACHE:
        _CACHE["nc"] = _build_nc()
    if "run" not in _CACHE:
        _CACHE["run"] = _build_sharded(_CACHE["nc"])
    in_maps = _prep_inputs(features, captions, emb_table, W_ih, W_hh, b_ih, b_hh,
                           fc_W, fc_b)
    results = _CACHE["run"](in_maps)
    return _unshard(results)


def kernel_traced(features, captions, emb_table, W_ih, W_hh, b_ih, b_hh, fc_W, fc_b):
    """Same computation via run_bass_kernel_spmd(trace=True); returns
    (output, BassKernelResults) so the caller can read exec_time_ns."""
    from concourse.bass_utils import run_bass_kernel_spmd
    if "nc" not in _CACHE:
        _CACHE["nc"] = _build_nc()
    in_maps = _prep_inputs(features, captions, emb_table, W_ih, W_hh, b_ih, b_hh,
                           fc_W, fc_b)
    res = run_bass_kernel_spmd(_CACHE["nc"], in_maps, list(range(NCORES)), trace=True)
    return _unshard(res.results), res


# revision 8
# speedup vs baseline: 1.3414x; 1.0302x over previous
"""Trainium2 Bass kernel for nn_Decoder (embed -> LSTM -> vocab projection).

v3.2 layout (8 NeuronCores, single SPMD NEFF):
  - Host: embedding gather + concat -> lstm_in; weights pre-transposed AND
    pre-packed so every preload DMA is partition-contiguous (fat
    descriptors; the naive rearranging loads were descriptor-rate-bound
    and delayed the first matmul to 43us).
  - LSTM is tensor-parallel over the HIDDEN dim: each core owns 128
    hidden units (512 gate rows i|f|g|o) and the full batch (256).
    Gates are computed TRANSPOSED, gates_T[512, 256], so every matmul
    runs at full M=128 PE efficiency, per-gate bias rides the ScalarE
    activation bias port, and h_T[128, 256] falls out of the elementwise
    ops already transposed (no PE transposes).
  - gx = x@W_ih^T contribution for t=1..9 is precomputed up front: ~31us
    of AllGather-independent PE work that fills the collective
    cold-start window. Steps then do identity-inject + 8 h-matmuls.
  - Warm-up AllGather on a host-supplied zero tensor absorbs the ~45us
    collective cold-start in the background.
  - Per-step AllGather of h_T chunks (64KB/core) rebuilds h_T[1024,256]
    on every core; next step's moving operand and the FC lhsT come
    straight from the gathered tile.
  - FC vocab-sharded: logits[2560, 3750] = hs @ fc_W_shard^T + fc_b,
    interleaved one step behind the recurrence (hides the ~8us AG).
    fc_W windows stream in consumption order via a 1-element-overlap
    WAW chain. Logits written bf16, one fat DMA per 128-row tile.
"""
import ml_dtypes
import numpy as np
import jax
from jax.sharding import Mesh, PartitionSpec
from jax.experimental.shard_map import shard_map

import concourse.bass as bass
import concourse.mybir as mybir
import concourse.tile as tile
from concourse import bacc
from concourse.bass2jax import _bass_exec_p, install_neuronx_cc_hook, partition_id_tensor
from concourse.masks import make_identity

P = 128
NCORES = 8
B, T, FEAT, EMB, HID, VOCAB = 256, 10, 512, 512, 1024, 30000
DIN = FEAT + EMB          # 1024
KX = DIN // P             # 8 x-contraction tiles
KH = HID // P             # 8 h-contraction tiles
KT = KX + KH              # 16
HL = HID // NCORES        # 128 hidden units per core
GC = 4 * HL               # 512 gate rows per core (i|f|g|o)
VL = VOCAB // NCORES      # 3750 vocab per core
NW = (VL + 511) // 512    # 8 fc windows
WSZ = KH * 512            # 4096 flat cols per (padded) fc window
RA = B * T                # 2560 logit rows (r = t*B + b)
XT = KX * B               # 2048 flat x cols per timestep
F32 = mybir.dt.float32
BF16 = mybir.dt.bfloat16
Act = mybir.ActivationFunctionType

_CACHE = {}


def _build_nc():
    nc = bacc.Bacc("TRN2", target_bir_lowering=False, debug=False, num_devices=NCORES)
    x_T = nc.dram_tensor("x_T", [P, T * XT], BF16, kind="ExternalInput").ap()
    wc_T = nc.dram_tensor("wc_T", [P, KT * GC], BF16, kind="ExternalInput").ap()
    bias4 = nc.dram_tensor("bias4", [P, 4], F32, kind="ExternalInput").ap()
    fc_w = nc.dram_tensor("fc_w", [P, NW * WSZ], BF16, kind="ExternalInput").ap()
    fc_b_rep = nc.dram_tensor("fc_b_rep", [P, VL], F32, kind="ExternalInput").ap()
    logits = nc.dram_tensor("logits", [RA, VL], BF16, kind="ExternalOutput").ap()

    GFUNC = [Act.Sigmoid, Act.Sigmoid, Act.Tanh, Act.Sigmoid]  # i, f, g, o

    with tile.TileContext(nc) as tc:
        with tc.tile_pool(name="dram", bufs=1, space="DRAM") as dram_pool:
            h_loc = dram_pool.tile([T, P, B], BF16)
            ag_outs = [dram_pool.tile([NCORES * P, B], BF16,
                                      addr_space="Shared", name=f"ag_{t}")
                       for t in range(T)]
            warm_out = dram_pool.tile([NCORES * P, 16], BF16,
                                      addr_space="Shared", name="ag_warm")
            warm_in = dram_pool.tile([P, 16], BF16, name="ag_warm_in")

            with tc.tile_pool(name="persist", bufs=1) as persist, \
                 tc.tile_pool(name="hT_pool", bufs=3) as hT_pool, \
                 tc.tile_pool(name="act_pool", bufs=2) as act_pool, \
                 tc.tile_pool(name="fc_out", bufs=2) as fc_out, \
                 tc.tile_pool(name="gpsum", bufs=1, space="PSUM") as gpsum, \
                 tc.tile_pool(name="fpsum", bufs=3, space="PSUM") as fpsum:
                wc_sb = persist.tile([P, KT * GC], BF16)
                x_sb = persist.tile([P, T * XT], BF16)
                bias_sb = persist.tile([P, 4], F32)
                fcw_sb = persist.tile([P, NW * WSZ], BF16)
                fcb_sb = persist.tile([P, VL], F32)
                c_sb = persist.tile([P, B], F32)
                gx_sb = persist.tile([P, 4 * (T - 1) * B], BF16)
                ident_f = persist.tile([P, P], F32)
                ident_b = persist.tile([P, P], BF16)

                # --- warm-up AllGather: starts the collective-path init
                # (~45us) in the background. Input is uninitialized DRAM
                # scratch (collectives cannot read IO tensors); the gathered
                # bytes are never consumed.
                nc.gpsimd.collective_compute(
                    "AllGather", mybir.AluOpType.bypass,
                    replica_groups=[list(range(NCORES))],
                    ins=[warm_in.opt()], outs=[warm_out.opt()])

                make_identity(nc, ident_f[:])
                nc.vector.tensor_copy(ident_b[:], ident_f[:])

                # --- phase-1 preloads: recurrence inputs only (7.2MB).
                # fc weights are deferred so they can't steal HBM bandwidth.
                nc.sync.dma_start(wc_sb[:], wc_T)
                nc.scalar.dma_start(x_sb[:], x_T)
                nc.scalar.dma_start(bias_sb[:], bias4)

                def phase2_preloads():
                    nc.scalar.dma_start(fcb_sb[:], fc_b_rep)
                    for j in range(NW):
                        # 1-element overlap chains the windows (WAW) so they
                        # complete in the order the FC consumes them
                        s0 = max(0, j * WSZ - 1)
                        nc.scalar.dma_start(
                            fcw_sb[:, s0:(j + 1) * WSZ],
                            fc_w[:, s0:(j + 1) * WSZ])

                def wslice(k, q):
                    return wc_sb[:, k * GC + q * HL:k * GC + (q + 1) * HL]

                def xslice(t, k):
                    return x_sb[:, t * XT + k * B:t * XT + (k + 1) * B]

                hT_tiles = {}

                def fc_step(s):
                    hs = hT_tiles[s]
                    for mh in range(2):
                        r0 = s * B + mh * P
                        ot = fc_out.tile([P, VL], BF16,
                                         name=f"fo_{s}_{mh}", tag="fo")
                        for j in range(NW):
                            n0 = j * 512
                            nsz = min(512, VL - n0)
                            ps = fpsum.tile([P, 512], F32,
                                            name=f"fps_{s}_{mh}_{j}", tag="fps")
                            for k in range(KH):
                                nc.tensor.matmul(
                                    ps[:, 0:nsz],
                                    hs[:, k, mh * P:(mh + 1) * P],
                                    fcw_sb[:, j * WSZ + k * 512:
                                           j * WSZ + k * 512 + nsz],
                                    start=(k == 0), stop=(k == KH - 1))
                            nc.vector.tensor_add(
                                ot[:, n0:n0 + nsz], ps[:, 0:nsz],
                                fcb_sb[:, n0:n0 + nsz])
                        nc.scalar.dma_start(logits[r0:r0 + P, :], ot[:])

                # --- step 0: x-part only (h=c=0; f-gate unused), acts
                # straight from PSUM with bias
                act4 = act_pool.tile([P, 4, B], F32, name="act_0", tag="act")
                for q in (0, 2, 3):
                    ps = gpsum.tile([P, B], F32, name=f"gps_0_{q}", tag=f"g{q}")
                    for k in range(KX):
                        nc.tensor.matmul(ps[:], wslice(k, q), xslice(0, k),
                                         start=(k == 0), stop=(k == KX - 1))
                    nc.scalar.activation(act4[:, q, :], ps[:], GFUNC[q],
                                         bias=bias_sb[:, q:q + 1])
                th = act_pool.tile([P, B], F32, name="th_0", tag="th")
                h_sb = act_pool.tile([P, B], BF16, name="h_0", tag="h")
                nc.vector.tensor_mul(c_sb[:], act4[:, 0, :], act4[:, 2, :])
                nc.scalar.activation(th[:], c_sb[:], Act.Tanh)
                nc.vector.tensor_mul(h_sb[:], act4[:, 3, :], th[:])
                nc.sync.dma_start(h_loc[0], h_sb[:])
                nc.gpsimd.collective_compute(
                    "AllGather", mybir.AluOpType.bypass,
                    replica_groups=[list(range(NCORES))],
                    ins=[h_loc[0].opt()], outs=[ag_outs[0].opt()])
                hT = hT_pool.tile([P, KH, B], BF16, name="hT_0", tag="hT")
                nc.sync.dma_start(
                    hT[:], ag_outs[0].rearrange("(j p) b -> p j b", p=P))
                hT_tiles[0] = hT

                phase2_preloads()

                # --- gx precompute for t=1..9: AG-independent PE work that
                # fills the collective cold-start window. Bias folded here.
                def gxslice(t, q):
                    o = (q * (T - 1) + (t - 1)) * B
                    return gx_sb[:, o:o + B]

                for t in range(1, T):
                    for q in range(4):
                        ps = gpsum.tile([P, B], F32,
                                        name=f"gxps_{t}_{q}", tag=f"g{q}")
                        for k in range(KX):
                            nc.tensor.matmul(ps[:], wslice(k, q), xslice(t, k),
                                             start=(k == 0), stop=(k == KX - 1))
                        nc.scalar.activation(gxslice(t, q), ps[:], Act.Identity,
                                             bias=bias_sb[:, q:q + 1])

                # --- steps 1..9: identity-inject gx, add h-part, then FC
                # for step t-1 backfills the PE while the AG flies
                for t in range(1, T):
                    act4 = act_pool.tile([P, 4, B], F32,
                                         name=f"act_{t}", tag="act")
                    for q in range(4):
                        ps = gpsum.tile([P, B], F32,
                                        name=f"gps_{t}_{q}", tag=f"g{q}")
                        nc.tensor.matmul(ps[:], ident_b[:], gxslice(t, q),
                                         start=True, stop=False)
                        for k in range(KH):
                            nc.tensor.matmul(
                                ps[:], wslice(KX + k, q), hT_tiles[t - 1][:, k, :],
                                start=False, stop=(k == KH - 1))
                        nc.scalar.activation(act4[:, q, :], ps[:], GFUNC[q])
                    th = act_pool.tile([P, B], F32, name=f"th_{t}", tag="th")
                    h_sb = act_pool.tile([P, B], BF16, name=f"h_{t}", tag="h")
                    tmp = act_pool.tile([P, B], F32, name=f"tmp_{t}", tag="tmp")
                    nc.vector.tensor_mul(tmp[:], act4[:, 0, :], act4[:, 2, :])
                    nc.vector.tensor_mul(c_sb[:], act4[:, 1, :], c_sb[:])
                    nc.vector.tensor_add(c_sb[:], c_sb[:], tmp[:])
                    nc.scalar.activation(th[:], c_sb[:], Act.Tanh)
                    nc.vector.tensor_mul(h_sb[:], act4[:, 3, :], th[:])
                    nc.sync.dma_start(h_loc[t], h_sb[:])
                    nc.gpsimd.collective_compute(
                        "AllGather", mybir.AluOpType.bypass,
                        replica_groups=[list(range(NCORES))],
                        ins=[h_loc[t].opt()], outs=[ag_outs[t].opt()])
                    hT = hT_pool.tile([P, KH, B], BF16,
                                      name=f"hT_{t}", tag="hT")
                    nc.sync.dma_start(
                        hT[:], ag_outs[t].rearrange("(j p) b -> p j b", p=P))
                    hT_tiles[t] = hT
                    fc_step(t - 1)
                fc_step(T - 1)
    nc.compile()
    return nc


def _build_sharded(nc, n_cores=NCORES):
    install_neuronx_cc_hook()
    partition_name = nc.partition_id_tensor.name if nc.partition_id_tensor else None
    in_names, out_names, out_avals, zero_shapes = [], [], [], []
    for alloc in nc.m.functions[0].allocations:
        if not isinstance(alloc, mybir.MemoryLocationSet):
            continue
        name = alloc.memorylocations[0].name
        if alloc.kind == "ExternalInput":
            if name != partition_name:
                in_names.append(name)
        elif alloc.kind == "ExternalOutput":
            out_names.append(name)
            shape = tuple(alloc.tensor_shape)
            dtype = mybir.dt.np(alloc.dtype)
            out_avals.append(jax.core.ShapedArray(shape, dtype))
            zero_shapes.append((shape, dtype))
    n_params = len(in_names)
    n_outs = len(out_avals)
    all_in_names = list(in_names) + list(out_names)
    if partition_name is not None:
        all_in_names.append(partition_name)
    donate = tuple(range(n_params, n_params + n_outs))

    def _body(*args):
        operands = list(args)
        if partition_name is not None:
            operands.append(partition_id_tensor())
        outs = _bass_exec_p.bind(
            *operands,
            out_avals=tuple(out_avals),
            in_names=tuple(all_in_names),
            out_names=tuple(out_names),
            lowering_input_output_aliases=(),
            sim_require_finite=True,
            sim_require_nnan=True,
            nc=nc,
        )
        return tuple(outs)

    devices = jax.devices("axon")[:n_cores]
    mesh = Mesh(np.asarray(devices), ("core",))
    in_specs = (PartitionSpec("core"),) * (n_params + n_outs)
    out_specs = (PartitionSpec("core"),) * len(out_names)
    sharded = jax.jit(
        shard_map(_body, mesh=mesh, in_specs=in_specs, out_specs=out_specs,
                  check_rep=False),
        donate_argnums=donate, keep_unused=True)

    def run(in_maps):
        concat_in = [
            np.concatenate([np.asarray(m[name]) for m in in_maps], axis=0)
            for name in in_names
        ]
        concat_zeros = [np.zeros((n_cores * s[0], *s[1:]), d) for s, d in zero_shapes]
        out_arrs = sharded(*concat_in, *concat_zeros)
        jax.block_until_ready(out_arrs)
        return [
            {name: np.asarray(out_arrs[i]).reshape(n_cores, *out_avals[i].shape)[c]
             for i, name in enumerate(out_names)}
            for c in range(n_cores)
        ]

    return run


def _prep_inputs(features, captions, emb_table, W_ih, W_hh, b_ih, b_hh, fc_W, fc_b):
    features = np.asarray(features, dtype=np.float32)
    captions = np.asarray(captions)
    emb_table = np.asarray(emb_table, dtype=np.float32)
    W_ih = np.asarray(W_ih, dtype=np.float32)
    W_hh = np.asarray(W_hh, dtype=np.float32)
    b = (np.asarray(b_ih, dtype=np.float32) + np.asarray(b_hh, dtype=np.float32))
    fc_W = np.asarray(fc_W, dtype=np.float32)
    fc_b = np.asarray(fc_b, dtype=np.float32)

    embedded = emb_table[captions.astype(np.int64)]          # [B, T, EMB]
    lstm_in = np.concatenate([features, embedded], axis=-1)  # [B, T, DIN]
    # x_host[p, t, k, b] = lstm_in[b, t, k*P+p]  -> flat [P, T*KX*B]
    x_host = np.ascontiguousarray(
        lstm_in.transpose(2, 1, 0)                           # [DIN, T, B]
        .reshape(KX, P, T, B).transpose(1, 2, 0, 3)          # [P, T, KX, B]
        .reshape(P, T * XT).astype(ml_dtypes.bfloat16))
    wcat = np.concatenate([W_ih, W_hh], axis=1)              # [4H, 2048]
    wcat4 = wcat.reshape(4, HID, KT * P)
    b4 = b.reshape(4, NCORES, HL)

    in_maps = []
    for c in range(NCORES):
        # wc_host[p, k, q*HL+j] = wcat[q*HID + c*HL + j, k*P+p]
        wc = wcat4[:, c * HL:(c + 1) * HL, :]                # [4, HL, 2048]
        wc_host = np.ascontiguousarray(
            wc.transpose(2, 0, 1).reshape(KT, P, GC)         # [KT, P, GC]
            .transpose(1, 0, 2).reshape(P, KT * GC).astype(ml_dtypes.bfloat16))
        bias4 = np.ascontiguousarray(b4[:, c, :].T)          # [HL, 4] f32
        # fc window-major padded: fcw_host[p, j, k, 0:nsz]
        fw = fc_W[c * VL:(c + 1) * VL].T                     # [HID, VL]
        fw_pk = fw.reshape(KH, P, VL).transpose(1, 0, 2)     # [P, KH, VL]
        fcw_host = np.zeros((P, NW, KH, 512), np.float32)
        for j in range(NW):
            n0 = j * 512
            nsz = min(512, VL - n0)
            fcw_host[:, j, :, 0:nsz] = fw_pk[:, :, n0:n0 + nsz]
        fcw_host = np.ascontiguousarray(
            fcw_host.reshape(P, NW * WSZ).astype(ml_dtypes.bfloat16))
        fcb_rep = np.ascontiguousarray(
            np.broadcast_to(fc_b[c * VL:(c + 1) * VL], (P, VL)))
        in_maps.append({
            "x_T": x_host, "wc_T": wc_host, "bias4": bias4,
            "fc_w": fcw_host, "fc_b_rep": fcb_rep,
        })
    return in_maps


def _unshard(results):
    out = np.empty((B, T, VOCAB), dtype=np.float32)
    for c in range(NCORES):
        lg = np.asarray(results[c]["logits"]).astype(np.float32)  # [RA, VL]
        out[:, :, c * VL:(c + 1) * VL] = lg.reshape(T, B, VL).transpose(1, 0, 2)
    return out


def kernel(features, captions, emb_table, W_ih, W_hh, b_ih, b_hh, fc_W, fc_b):
    if "nc" not in _CACHE:
        _CACHE["nc"] = _build_nc()
    if "run" not in _CACHE:
        _CACHE["run"] = _build_sharded(_CACHE["nc"])
    in_maps = _prep_inputs(features, captions, emb_table, W_ih, W_hh, b_ih, b_hh,
                           fc_W, fc_b)
    results = _CACHE["run"](in_maps)
    return _unshard(results)


def kernel_traced(features, captions, emb_table, W_ih, W_hh, b_ih, b_hh, fc_W, fc_b):
    """Same computation via run_bass_kernel_spmd(trace=True); returns
    (output, BassKernelResults) so the caller can read exec_time_ns."""
    from concourse.bass_utils import run_bass_kernel_spmd
    if "nc" not in _CACHE:
        _CACHE["nc"] = _build_nc()
    in_maps = _prep_inputs(features, captions, emb_table, W_ih, W_hh, b_ih, b_hh,
                           fc_W, fc_b)
    res = run_bass_kernel_spmd(_CACHE["nc"], in_maps, list(range(NCORES)), trace=True)
    return _unshard(res.results), res
